# revision 21
# baseline (speedup 1.0000x reference)
"""Trainium2 Bass kernel for nn_MlpwithSOMModule (retrieval_knn).

Reference computation, per (b, k) pair:
    ctx, ent = context[b,k,0], context[b,k,1]        # [L=128, D=768]
    sim      = ctx @ ent.T                           # [128, 128]
    idx      = argmax(sim, -1)
    x        = concat([ctx, ent[idx]], -1)           # [128, 1536]
    7-layer tanh MLP (1536->768->384->192->96->48->24->1), out = sum_l x7[l]

Strategy: data-parallel over the 1024 (b,k) pairs -> 128 pairs per core.
Host pre-transposes ctx/ent to feature-major fp16 tiles (the PE contracts
over the partition dim, so both sim and the MLP want feature-on-partition
layouts); ent is additionally shipped in natural layout as the stationary
operand of a one-hot gather matmul.  All matmuls run in fp16 (1 cycle/row
on TRN2 vs 4 for fp32) with fp32 PSUM accumulation; biases+tanh are fused
on the scalar engine in fp32.  The MLP processes 4 pairs per matmul group
(moving free dim N=512) to amortize weight loads.
"""

import sys

import numpy as np

for _p in ("/opt/trn_rl_repo", "/root/.axon_site/_ro/trn_rl_repo"):
    if _p not in sys.path:
        sys.path.append(_p)

import concourse.mybir as mybir  # noqa: E402
import concourse.tile as tile  # noqa: E402
from concourse import bacc  # noqa: E402
from concourse.bass_utils import run_bass_kernel_spmd  # noqa: E402
from concourse.masks import make_identity  # noqa: E402

B, K, L, D = 16, 64, 128, 768
NCORES = 8
NPAIR = (B * K) // NCORES  # 128 pairs per core
G = 4  # pairs per MLP group
NG = NPAIR // G
NDC = D // 128  # 6 feature chunks of ctx/ent

# (din, dout) per layer; first layer input is concat(ctx, gathered) = 2D
LAYER_DIMS = [(2 * D, 768), (768, 384), (384, 192), (192, 96), (96, 48), (48, 24), (24, 1)]
F16 = mybir.dt.float16
F32 = mybir.dt.float32
F8 = mybir.dt.float8e4
DR_SCALE = 64.0


def _chunks(n):
    """Split n into partition chunks of <=128."""
    out = []
    while n > 0:
        out.append(min(n, 128))
        n -= 128
    return out


KC_SIZES = [_chunks(din) for din, _ in LAYER_DIMS]
MC_SIZES = [_chunks(dout) for _, dout in LAYER_DIMS]


def build_nc(ng: int = NG, reps: int = 1, do_pair: bool = True, do_mlp: bool = True, mm_pairs: int = G, n_dr: int = 2):
    """Emit the Bass/Tile program (identical on all 8 cores).

    reps > 1 wraps the whole compute in a hardware loop that recomputes the
    same output; used only for steady-state timing (amortizes the large
    axon/PJRT per-dispatch overhead out of the measurement).
    """
    nc = bacc.Bacc("TRN2", target_bir_lowering=False, debug=False, num_devices=NCORES)

    ctxT_d = nc.dram_tensor("ctxT", [NPAIR, 128, D], F16, kind="ExternalInput")
    entT_d = nc.dram_tensor("entT", [NPAIR, 128, D], F16, kind="ExternalInput")
    entn_d = nc.dram_tensor("entn", [NPAIR, 128, D], F16, kind="ExternalInput")
    w_d = []
    b_d = []
    for i, (din, dout) in enumerate(LAYER_DIMS):
        nkc = len(KC_SIZES[i])
        w_d.append(nc.dram_tensor(f"w{i}", [nkc, 128, dout], F16, kind="ExternalInput"))
        if i < 6:
            nmc = len(MC_SIZES[i])
            b_d.append(nc.dram_tensor(f"b{i}", [nmc, 128], F32, kind="ExternalInput"))
    wdr_d = [
        nc.dram_tensor("w0dr", [6, 128, 2, 768], F8, kind="ExternalInput"),
        nc.dram_tensor("w1dr", [3, 128, 2, 384], F8, kind="ExternalInput"),
    ]
    b7s_d = nc.dram_tensor("b7s", [1, 1], F32, kind="ExternalInput")
    out_d = nc.dram_tensor("out", [1, NPAIR], F32, kind="ExternalOutput")

    with tile.TileContext(nc) as tc:
        with (
            tc.tile_pool(name="const", bufs=1) as cpool,
            tc.tile_pool(name="xg", bufs=3) as xgpool,
            tc.tile_pool(name="ys", bufs=2) as ypool,
            tc.tile_pool(name="pair", bufs=8) as ppool,
            tc.tile_pool(name="small", bufs=4) as spool,
            tc.tile_pool(name="outp", bufs=1) as opool,
            tc.tile_pool(name="ps_sim", bufs=2, space="PSUM") as pp_sim,
            tc.tile_pool(name="ps_oh", bufs=1, space="PSUM") as pp_oh,
            tc.tile_pool(name="ps_g", bufs=2, space="PSUM") as pp_g,
            tc.tile_pool(name="ps_mlp", bufs=2, space="PSUM") as pp_mlp,
            tc.tile_pool(name="ps_y7", bufs=1, space="PSUM") as pp_y7,
        ):
            ident = cpool.tile([128, 128], F16)
            make_identity(nc, ident[:])

            w_sb = []
            b_sb = []
            for i, (din, dout) in enumerate(LAYER_DIMS):
                nkc = len(KC_SIZES[i])
                w = cpool.tile([128, nkc, dout], F16, tag=f"w{i}", name=f"w{i}")
                if i >= n_dr:
                    nc.sync.dma_start(w[:], w_d[i].rearrange("k p d -> p k d"))
                w_sb.append(w)
                if i < 6:
                    nmc = len(MC_SIZES[i])
                    b = cpool.tile([128, nmc], F32, tag=f"b{i}", name=f"bb{i}")
                    nc.sync.dma_start(b[:], b_d[i].rearrange("c p -> p c"))
                    b_sb.append(b)
            wdr_sb = [
                cpool.tile([128, 6, 2, 768], F8, tag="w0dr", name="w0dr"),
                cpool.tile([128, 3, 2, 384], F8, tag="w1dr", name="w1dr"),
            ]
            for i in range(n_dr):
                nc.sync.dma_start(wdr_sb[i][:], wdr_d[i].rearrange("c p j d -> p c j d"))
            b7s = cpool.tile([1, 1], F32)
            nc.sync.dma_start(b7s[:], b7s_d[:])

            out_sb = opool.tile([1, NPAIR], F32)

            def lhsT(i, kc, mc):
                kcs = KC_SIZES[i][kc]
                mco = mc * 128
                mcs = MC_SIZES[i][mc]
                return w_sb[i][0:kcs, kc, mco : mco + mcs]

            def emit_pair_phase(g):
                # xg free layout: (pair, kchunk, l); rhs slice for MLP kc is
                # xg[:, :, kc, :] -> [128, G, 128] = N=512 moving operand
                xg = xgpool.tile([128, G, 12, 128], F8 if n_dr > 0 else F16)
                ctxT = [ppool.tile([128, NDC, 128], F16, tag="ctxT", name=f"ctxT{pi}") for pi in range(G)]
                entT = [ppool.tile([128, NDC, 128], F16, tag="entT", name=f"entT{pi}") for pi in range(G)]
                entn = [ppool.tile([128, D], F16, tag="entn", name=f"entn{pi}") for pi in range(G)]

                for pi in range(G):
                    p = g * G + pi
                    nc.sync.dma_start(ctxT[pi][:], ctxT_d[p].rearrange("p (c l) -> p c l", c=NDC))
                    nc.sync.dma_start(entT[pi][:], entT_d[p].rearrange("p (c l) -> p c l", c=NDC))
                    nc.sync.dma_start(entn[pi][:], entn_d[p])
                    for c in range(NDC):
                        nc.vector.tensor_copy(xg[:, pi, c, :], ctxT[pi][:, c, :])

                if not do_pair:
                    return xg
                # similarity: sim[l, m] = sum_d ctxT[d, l] * entT[d, m];
                # per-pair argmax one-hot on DVE right behind each sim
                oh_lms = []
                for pi in range(G):
                    ps = pp_sim.tile([128, 128], F32, tag="psim")
                    for c in range(NDC):
                        nc.tensor.matmul(
                            ps[:],
                            ctxT[pi][:, c, :],
                            entT[pi][:, c, :],
                            start=(c == 0),
                            stop=(c == NDC - 1),
                        )
                    sim_sb = spool.tile([128, 128], F32, tag="sim")
                    nc.vector.tensor_copy(sim_sb[:], ps[:])
                    maxs = spool.tile([128, 8], F32, tag="maxs")
                    nc.vector.max(out=maxs[:], in_=sim_sb[:])
                    oh_lm = spool.tile([128, 128], F16, tag="ohlm")
                    nc.vector.tensor_scalar(
                        oh_lm[:], sim_sb[:], maxs[:, 0:1], None, op0=mybir.AluOpType.is_equal
                    )
                    oh_lms.append(oh_lm)

                for pi in range(G):
                    oh_lm = oh_lms[pi]
                    poh = pp_oh.tile([128, 128], F16, tag="poh")
                    nc.tensor.transpose(poh[:], oh_lm[:], ident[:])
                    oh_ml = spool.tile([128, 128], F16, tag="ohml")
                    nc.vector.tensor_copy(oh_ml[:], poh[:])
                    # gathered.T chunk = ent_nat[:, chunk].T @ onehot
                    for c in range(NDC):
                        pg = pp_g.tile([128, 128], F32, tag="pg")
                        nc.tensor.matmul(
                            pg[:],
                            entn[pi][:, c * 128 : (c + 1) * 128],
                            oh_ml[:],
                            start=True,
                            stop=True,
                        )
                        nc.vector.tensor_copy(xg[:, pi, NDC + c, :], pg[:])
                return xg

            def emit_mlp(g, xg):
                if not do_mlp:
                    return
                # MLP on the group of G pairs (N = G*128 = 512)
                yprev = xg
                for i in range(6):
                    dout = LAYER_DIMS[i][1]
                    nkc_out = len(_chunks(dout))
                    ydt = F8 if (i + 1) < n_dr else F16
                    y = ypool.tile([128, G, nkc_out, 128], ydt, tag=f"y{i}", name=f"y{i}")
                    for mc, mcs in enumerate(MC_SIZES[i]):
                        pm = pp_mlp.tile([128, G, 128], F32, tag="pmlp")
                        if i < n_dr:
                            nc2 = LAYER_DIMS[i][0] // 256
                            for c2 in range(nc2):
                                nc.tensor.matmul(
                                    pm[:, :, :],
                                    wdr_sb[i][:, c2, :, mc * 128 : (mc + 1) * 128],
                                    yprev[:, :, 2 * c2 : 2 * c2 + 2, :].rearrange("p g j l -> p j g l"),
                                    start=(c2 == 0),
                                    stop=(c2 == nc2 - 1),
                                    perf_mode=mybir.MatmulPerfMode.DoubleRow,
                                )
                        else:
                            for p0 in range(0, G, mm_pairs):
                                for kc, kcs in enumerate(KC_SIZES[i]):
                                    nc.tensor.matmul(
                                        pm[0:mcs, p0 : p0 + mm_pairs, :],
                                        lhsT(i, kc, mc),
                                        yprev[0:kcs, p0 : p0 + mm_pairs, kc, :],
                                        start=(kc == 0),
                                        stop=(kc == len(KC_SIZES[i]) - 1),
                                    )
                        nc.scalar.activation(
                            y[0:mcs, :, mc, :],
                            pm[0:mcs, :, :],
                            mybir.ActivationFunctionType.Tanh,
                            bias=b_sb[i][0:mcs, mc : mc + 1],
                            scale=(1.0 / DR_SCALE) if i < n_dr else 1.0,
                        )
                    yprev = y

                # final layer 24 -> 1, then reduce over l within each pair
                py7 = pp_y7.tile([1, G, 128], F32)
                nc.tensor.matmul(
                    py7[:], lhsT(6, 0, 0), yprev[0:24, :, 0, :], start=True, stop=True
                )
                nc.vector.tensor_reduce(
                    out_sb[0:1, g * G : (g + 1) * G],
                    py7[:],
                    axis=mybir.AxisListType.X,
                    op=mybir.AluOpType.add,
                )

            def emit_body():
                # software pipeline: pair-phase of group g+1 is emitted (and
                # runs on PE/DVE) ahead of group g's MLP, so the DVE argmax
                # chain never stalls the PE
                xg_prev = emit_pair_phase(0)
                for g in range(ng):
                    if g + 1 < ng:
                        xg_next = emit_pair_phase(g + 1)
                    emit_mlp(g, xg_prev)
                    if g + 1 < ng:
                        xg_prev = xg_next
                nc.vector.tensor_scalar(
                    out_sb[:], out_sb[:], b7s[0:1, 0:1], None, op0=mybir.AluOpType.add
                )
                nc.sync.dma_start(out_d[:], out_sb[:])

            if reps == 1:
                emit_body()
            else:
                with tc.For_i(0, reps, 1):
                    emit_body()

    nc.compile()
    return nc


def prep_in_maps(context, Ws, bs):
    """Shard + relayout the full inputs into per-core input dicts."""
    context = np.asarray(context)
    ctx = context[:, :, 0].reshape(B * K, L, D)
    ent = context[:, :, 1].reshape(B * K, L, D)

    def featmajor(a):  # [P, l, d] -> [P, 128(dpart), 6(dchunk), 128(l)] as [P,128,D]
        return np.ascontiguousarray(
            a.reshape(-1, L, NDC, 128).transpose(0, 3, 2, 1)
        ).astype(np.float16).reshape(-1, 128, D)

    ctxT = featmajor(ctx)
    entT = featmajor(ent)
    entn = np.ascontiguousarray(ent).astype(np.float16)

    common = {}
    for i, (din, dout) in enumerate(LAYER_DIMS):
        nkc = len(KC_SIZES[i])
        wt = np.zeros((nkc * 128, dout), np.float32)
        wt[:din] = np.asarray(Ws[i]).T
        common[f"w{i}"] = wt.reshape(nkc, 128, dout).astype(np.float16)
        if i < 6:
            nmc = len(MC_SIZES[i])
            bt = np.zeros((nmc * 128,), np.float32)
            bt[:dout] = np.asarray(bs[i])
            common[f"b{i}"] = bt.reshape(nmc, 128)
    import ml_dtypes

    for i in range(2):
        din, dout = LAYER_DIMS[i]
        wt = np.asarray(Ws[i]).T.astype(np.float32) * DR_SCALE
        common[f"w{i}dr"] = np.ascontiguousarray(
            wt.reshape(din // 256, 2, 128, dout).transpose(0, 2, 1, 3)
        ).astype(ml_dtypes.float8_e4m3)
    common["b7s"] = np.full((1, 1), float(np.asarray(bs[6])[0]) * L, np.float32)

    in_maps = []
    for c in range(NCORES):
        s = slice(c * NPAIR, (c + 1) * NPAIR)
        in_maps.append(
            {"ctxT": ctxT[s], "entT": entT[s], "entn": entn[s], **common}
        )
    return in_maps


def kernel(context, Ws, bs):
    in_maps = prep_in_maps(context, Ws, bs)
    nc = build_nc()
    res = run_bass_kernel_spmd(nc, in_maps, core_ids=list(range(NCORES)))
    out = np.concatenate([r["out"].reshape(NPAIR) for r in res.results])
    return out.reshape(B, K).astype(np.float32)


if __name__ == "__main__":
    rng = np.random.default_rng(0)
    context = rng.standard_normal((B, K, 2, L, D), dtype=np.float32)
    Ws = [rng.standard_normal((dout, din), dtype=np.float32) * 0.02 for din, dout in LAYER_DIMS]
    bs = [rng.standard_normal((dout,), dtype=np.float32) * 0.02 for _, dout in LAYER_DIMS]
    out = kernel(context, tuple(Ws), tuple(bs))
    print("kernel out:", out.shape, out.dtype, out.reshape(-1)[:4])


# revision 27
# speedup vs baseline: 1.0932x; 1.0932x over previous
"""Trainium2 Bass kernel for nn_MlpwithSOMModule (retrieval_knn).

Reference computation, per (b, k) pair:
    ctx, ent = context[b,k,0], context[b,k,1]        # [L=128, D=768]
    sim      = ctx @ ent.T                           # [128, 128]
    idx      = argmax(sim, -1)
    x        = concat([ctx, ent[idx]], -1)           # [128, 1536]
    7-layer tanh MLP (1536->768->384->192->96->48->24->1), out = sum_l x7[l]

Strategy: data-parallel over the 1024 (b,k) pairs -> 128 pairs per core.
Host pre-transposes ctx/ent to feature-major fp16 tiles (the PE contracts
over the partition dim, so both sim and the MLP want feature-on-partition
layouts); ent is additionally shipped in natural layout as the stationary
operand of a one-hot gather matmul.  All matmuls run in fp16 (1 cycle/row
on TRN2 vs 4 for fp32) with fp32 PSUM accumulation; biases+tanh are fused
on the scalar engine in fp32.  The MLP processes 4 pairs per matmul group
(moving free dim N=512) to amortize weight loads.
"""

import sys

import numpy as np

for _p in ("/opt/trn_rl_repo", "/root/.axon_site/_ro/trn_rl_repo"):
    if _p not in sys.path:
        sys.path.append(_p)

import concourse.mybir as mybir  # noqa: E402
import concourse.tile as tile  # noqa: E402
from concourse import bacc  # noqa: E402
from concourse.bass_utils import run_bass_kernel_spmd  # noqa: E402
from concourse.masks import make_identity  # noqa: E402

B, K, L, D = 16, 64, 128, 768
NCORES = 8
NPAIR = (B * K) // NCORES  # 128 pairs per core
G = 4  # pairs per MLP group
NG = NPAIR // G
NDC = D // 128  # 6 feature chunks of ctx/ent

# (din, dout) per layer; first layer input is concat(ctx, gathered) = 2D
LAYER_DIMS = [(2 * D, 768), (768, 384), (384, 192), (192, 96), (96, 48), (48, 24), (24, 1)]
F16 = mybir.dt.float16
F32 = mybir.dt.float32
F8 = mybir.dt.float8e4
DR_SCALE = 64.0


def _chunks(n):
    """Split n into partition chunks of <=128."""
    out = []
    while n > 0:
        out.append(min(n, 128))
        n -= 128
    return out


KC_SIZES = [_chunks(din) for din, _ in LAYER_DIMS]
MC_SIZES = [_chunks(dout) for _, dout in LAYER_DIMS]


def build_nc(ng: int = NG, reps: int = 1, do_pair: bool = True, do_mlp: bool = True, mm_pairs: int = G, n_dr: int = 2):
    """Emit the Bass/Tile program (identical on all 8 cores).

    reps > 1 wraps the whole compute in a hardware loop that recomputes the
    same output; used only for steady-state timing (amortizes the large
    axon/PJRT per-dispatch overhead out of the measurement).
    """
    nc = bacc.Bacc("TRN2", target_bir_lowering=False, debug=False, num_devices=NCORES)

    ctxT_d = nc.dram_tensor("ctxT", [NPAIR, 128, D], F16, kind="ExternalInput")
    entT_d = nc.dram_tensor("entT", [NPAIR, 128, D], F16, kind="ExternalInput")
    entn_d = nc.dram_tensor("entn", [NPAIR, 128, D], F16, kind="ExternalInput")
    w_d = []
    b_d = []
    for i, (din, dout) in enumerate(LAYER_DIMS):
        nkc = len(KC_SIZES[i])
        w_d.append(nc.dram_tensor(f"w{i}", [nkc, 128, dout], F16, kind="ExternalInput"))
        if i < 6:
            nmc = len(MC_SIZES[i])
            b_d.append(nc.dram_tensor(f"b{i}", [nmc, 128], F32, kind="ExternalInput"))
    wdr_d = [
        nc.dram_tensor("w0dr", [6, 128, 2, 768], F8, kind="ExternalInput"),
        nc.dram_tensor("w1dr", [3, 128, 2, 384], F8, kind="ExternalInput"),
    ]
    b7s_d = nc.dram_tensor("b7s", [1, 1], F32, kind="ExternalInput")
    out_d = nc.dram_tensor("out", [1, NPAIR], F32, kind="ExternalOutput")

    with tile.TileContext(nc) as tc:
        with (
            tc.tile_pool(name="const", bufs=1) as cpool,
            tc.tile_pool(name="xg", bufs=3) as xgpool,
            tc.tile_pool(name="ys", bufs=2) as ypool,
            tc.tile_pool(name="pair", bufs=8) as ppool,
            tc.tile_pool(name="small", bufs=4) as spool,
            tc.tile_pool(name="outp", bufs=1) as opool,
            tc.tile_pool(name="ps_sim", bufs=2, space="PSUM") as pp_sim,
            tc.tile_pool(name="ps_oh", bufs=1, space="PSUM") as pp_oh,
            tc.tile_pool(name="ps_g", bufs=2, space="PSUM") as pp_g,
            tc.tile_pool(name="ps_mlp", bufs=2, space="PSUM") as pp_mlp,
            tc.tile_pool(name="ps_y7", bufs=1, space="PSUM") as pp_y7,
        ):
            ident = cpool.tile([128, 128], F16)
            make_identity(nc, ident[:])

            w_sb = []
            b_sb = []
            for i, (din, dout) in enumerate(LAYER_DIMS):
                nkc = len(KC_SIZES[i])
                w = cpool.tile([128, nkc, dout], F16, tag=f"w{i}", name=f"w{i}")
                if i >= n_dr:
                    nc.sync.dma_start(w[:], w_d[i].rearrange("k p d -> p k d"))
                w_sb.append(w)
                if i < 6:
                    nmc = len(MC_SIZES[i])
                    b = cpool.tile([128, nmc], F32, tag=f"b{i}", name=f"bb{i}")
                    nc.sync.dma_start(b[:], b_d[i].rearrange("c p -> p c"))
                    b_sb.append(b)
            wdr_sb = [
                cpool.tile([128, 6, 2, 768], F8, tag="w0dr", name="w0dr"),
                cpool.tile([128, 3, 2, 384], F8, tag="w1dr", name="w1dr"),
            ]
            for i in range(n_dr):
                nc.sync.dma_start(wdr_sb[i][:], wdr_d[i].rearrange("c p j d -> p c j d"))
            b7s = cpool.tile([1, 1], F32)
            nc.sync.dma_start(b7s[:], b7s_d[:])

            out_sb = opool.tile([1, NPAIR], F32)

            def lhsT(i, kc, mc):
                kcs = KC_SIZES[i][kc]
                mco = mc * 128
                mcs = MC_SIZES[i][mc]
                return w_sb[i][0:kcs, kc, mco : mco + mcs]

            def emit_pair_phase(g):
                # xg free layout: (pair, kchunk, l); rhs slice for MLP kc is
                # xg[:, :, kc, :] -> [128, G, 128] = N=512 moving operand
                xg = xgpool.tile([128, G, 12, 128], F8 if n_dr > 0 else F16)
                ctxT = [ppool.tile([128, NDC, 128], F16, tag="ctxT", name=f"ctxT{pi}") for pi in range(G)]
                entT = [ppool.tile([128, NDC, 128], F16, tag="entT", name=f"entT{pi}") for pi in range(G)]
                entn = [ppool.tile([128, D], F16, tag="entn", name=f"entn{pi}") for pi in range(G)]

                for pi in range(G):
                    p = g * G + pi
                    nc.sync.dma_start(ctxT[pi][:], ctxT_d[p].rearrange("p (c l) -> p c l", c=NDC))
                    nc.sync.dma_start(entT[pi][:], entT_d[p].rearrange("p (c l) -> p c l", c=NDC))
                    nc.sync.dma_start(entn[pi][:], entn_d[p])
                    for c in range(NDC):
                        nc.vector.tensor_copy(xg[:, pi, c, :], ctxT[pi][:, c, :])

                if not do_pair:
                    return xg
                # similarity: sim[l, m] = sum_d ctxT[d, l] * entT[d, m];
                # per-pair argmax one-hot on DVE right behind each sim
                oh_lms = []
                for pi in range(G):
                    ps = pp_sim.tile([128, 128], F32, tag="psim")
                    for c in range(NDC):
                        nc.tensor.matmul(
                            ps[:],
                            ctxT[pi][:, c, :],
                            entT[pi][:, c, :],
                            start=(c == 0),
                            stop=(c == NDC - 1),
                        )
                    sim_sb = spool.tile([128, 128], F32, tag="sim")
                    nc.vector.tensor_copy(sim_sb[:], ps[:])
                    maxs = spool.tile([128, 8], F32, tag="maxs")
                    nc.vector.max(out=maxs[:], in_=sim_sb[:])
                    oh_lm = spool.tile([128, 128], F16, tag="ohlm")
                    nc.vector.tensor_scalar(
                        oh_lm[:], sim_sb[:], maxs[:, 0:1], None, op0=mybir.AluOpType.is_equal
                    )
                    oh_lms.append(oh_lm)

                for pi in range(G):
                    oh_lm = oh_lms[pi]
                    poh = pp_oh.tile([128, 128], F16, tag="poh")
                    nc.tensor.transpose(poh[:], oh_lm[:], ident[:])
                    oh_ml = spool.tile([128, 128], F16, tag="ohml")
                    nc.vector.tensor_copy(oh_ml[:], poh[:])
                    # gathered.T chunk = ent_nat[:, chunk].T @ onehot
                    for c in range(NDC):
                        pg = pp_g.tile([128, 128], F32, tag="pg")
                        nc.tensor.matmul(
                            pg[:],
                            entn[pi][:, c * 128 : (c + 1) * 128],
                            oh_ml[:],
                            start=True,
                            stop=True,
                        )
                        nc.vector.tensor_copy(xg[:, pi, NDC + c, :], pg[:])
                return xg

            def emit_mlp(g, xg):
                if not do_mlp:
                    return
                # MLP on the group of G pairs (N = G*128 = 512)
                yprev = xg
                for i in range(6):
                    dout = LAYER_DIMS[i][1]
                    nkc_out = len(_chunks(dout))
                    ydt = F8 if (i + 1) < n_dr else F16
                    y = ypool.tile([128, G, nkc_out, 128], ydt, tag=f"y{i}", name=f"y{i}")
                    for mc, mcs in enumerate(MC_SIZES[i]):
                        pm = pp_mlp.tile([128, G, 128], F32, tag="pmlp")
                        if i < n_dr:
                            nc2 = LAYER_DIMS[i][0] // 256
                            for c2 in range(nc2):
                                nc.tensor.matmul(
                                    pm[:, :, :],
                                    wdr_sb[i][:, c2, :, mc * 128 : (mc + 1) * 128],
                                    yprev[:, :, 2 * c2 : 2 * c2 + 2, :].rearrange("p g j l -> p j g l"),
                                    start=(c2 == 0),
                                    stop=(c2 == nc2 - 1),
                                    perf_mode=mybir.MatmulPerfMode.DoubleRow,
                                )
                        else:
                            for p0 in range(0, G, mm_pairs):
                                for kc, kcs in enumerate(KC_SIZES[i]):
                                    nc.tensor.matmul(
                                        pm[0:mcs, p0 : p0 + mm_pairs, :],
                                        lhsT(i, kc, mc),
                                        yprev[0:kcs, p0 : p0 + mm_pairs, kc, :],
                                        start=(kc == 0),
                                        stop=(kc == len(KC_SIZES[i]) - 1),
                                    )
                        nc.scalar.activation(
                            y[0:mcs, :, mc, :],
                            pm[0:mcs, :, :],
                            mybir.ActivationFunctionType.Tanh,
                            bias=b_sb[i][0:mcs, mc : mc + 1],
                            scale=(1.0 / DR_SCALE) if i < n_dr else 1.0,
                        )
                    yprev = y

                # final layer 24 -> 1, then reduce over l within each pair
                py7 = pp_y7.tile([1, G, 128], F32)
                nc.tensor.matmul(
                    py7[:], lhsT(6, 0, 0), yprev[0:24, :, 0, :], start=True, stop=True
                )
                nc.vector.tensor_reduce(
                    out_sb[0:1, g * G : (g + 1) * G],
                    py7[:],
                    axis=mybir.AxisListType.X,
                    op=mybir.AluOpType.add,
                )

            def emit_body():
                # software pipeline: pair-phase of group g+1 is emitted (and
                # runs on PE/DVE) ahead of group g's MLP, so the DVE argmax
                # chain never stalls the PE
                xg_prev = emit_pair_phase(0)
                for g in range(ng):
                    if g + 1 < ng:
                        xg_next = emit_pair_phase(g + 1)
                    emit_mlp(g, xg_prev)
                    if g + 1 < ng:
                        xg_prev = xg_next
                nc.vector.tensor_scalar(
                    out_sb[:], out_sb[:], b7s[0:1, 0:1], None, op0=mybir.AluOpType.add
                )
                nc.sync.dma_start(out_d[:], out_sb[:])

            if reps == 1:
                emit_body()
            else:
                with tc.For_i(0, reps, 1):
                    emit_body()

    nc.compile()
    return nc


def prep_in_maps(context, Ws, bs):
    """Shard + relayout the full inputs into per-core input dicts."""
    context = np.asarray(context)
    ctx = context[:, :, 0].reshape(B * K, L, D)
    ent = context[:, :, 1].reshape(B * K, L, D)

    def featmajor(a):  # [P, l, d] -> [P, 128(dpart), 6(dchunk), 128(l)] as [P,128,D]
        return np.ascontiguousarray(
            a.reshape(-1, L, NDC, 128).transpose(0, 3, 2, 1)
        ).astype(np.float16).reshape(-1, 128, D)

    ctxT = featmajor(ctx)
    entT = featmajor(ent)
    entn = np.ascontiguousarray(ent).astype(np.float16)

    common = {}
    for i, (din, dout) in enumerate(LAYER_DIMS):
        nkc = len(KC_SIZES[i])
        wt = np.zeros((nkc * 128, dout), np.float32)
        wt[:din] = np.asarray(Ws[i]).T
        common[f"w{i}"] = wt.reshape(nkc, 128, dout).astype(np.float16)
        if i < 6:
            nmc = len(MC_SIZES[i])
            bt = np.zeros((nmc * 128,), np.float32)
            bt[:dout] = np.asarray(bs[i])
            common[f"b{i}"] = bt.reshape(nmc, 128)
    import ml_dtypes

    for i in range(2):
        din, dout = LAYER_DIMS[i]
        wt = np.asarray(Ws[i]).T.astype(np.float32) * DR_SCALE
        common[f"w{i}dr"] = np.ascontiguousarray(
            wt.reshape(din // 256, 2, 128, dout).transpose(0, 2, 1, 3)
        ).astype(ml_dtypes.float8_e4m3)
    common["b7s"] = np.full((1, 1), float(np.asarray(bs[6])[0]) * L, np.float32)

    in_maps = []
    for c in range(NCORES):
        s = slice(c * NPAIR, (c + 1) * NPAIR)
        in_maps.append(
            {"ctxT": ctxT[s], "entT": entT[s], "entn": entn[s], **common}
        )
    return in_maps


def kernel(context, Ws, bs):
    in_maps = prep_in_maps(context, Ws, bs)
    nc = build_nc()
    res = run_bass_kernel_spmd(nc, in_maps, core_ids=list(range(NCORES)))
    out = np.concatenate([r["out"].reshape(NPAIR) for r in res.results])
    return out.reshape(B, K).astype(np.float32)


if __name__ == "__main__":
    rng = np.random.default_rng(0)
    context = rng.standard_normal((B, K, 2, L, D), dtype=np.float32)
    Ws = [rng.standard_normal((dout, din), dtype=np.float32) * 0.02 for din, dout in LAYER_DIMS]
    bs = [rng.standard_normal((dout,), dtype=np.float32) * 0.02 for _, dout in LAYER_DIMS]
    out = kernel(context, tuple(Ws), tuple(bs))
    print("kernel out:", out.shape, out.dtype, out.reshape(-1)[:4])
